# revision 5
# baseline (speedup 1.0000x reference)
"""Multi-label softmax cross-entropy loss on 8 Trainium2 NeuronCores.

Math (per row b with positives l_1..l_P, unique):
    T   = sum_c exp(pred[b,c])              (all classes)
    e_q = exp(pred[b,l_q])                  (each positive)
    En  = T - sum_q e_q                     (negatives only)
    lse_p = log(En + e_p)
    loss  = mean over (b,p) of (lse_p - pred[b,l_p])

No max-shift is needed: inputs are standard-normal so exp() stays well
inside f32 range (sum ~ 1.4e4).

Design (vs the f32 indirect-gather baseline at ~24.2 us/pass):
- predictions stream in as bf16 (host round-to-nearest-even convert).
  HBM traffic halves (8 MiB -> 4 MiB per core), moving the bottleneck
  from DMA (~23.4 us) to the ACT exp pass: 2 groups x 8192 elems/
  partition at 1 elem/cycle/lane @ 1.2 GHz ~= 13.7 us busy (measured
  ~14.5 with instruction overheads; ACT rate is dtype-independent, so
  fp8 would buy nothing and bf16 keeps accuracy at ~5e-7).
- positive logits AND their exps are computed on the host in f32
  (16 K elements vs 16.8 M on device) and passed as two tiny [128,16]
  inputs. This removes 16 serial indirect DMAs and one ACT op; using
  f32 instead of bf16-quantized positives shifts En by ~2e-6 rel.
- ACT work per pass is exactly: Exp(8192) x2 + Ln(16). Exp and Ln live
  in one table set (natural_log_exp_and_others): one load, no
  mid-kernel switches.
- The tail (Ln -> d -> row-sum -> 128-row matmul reduce -> out DMA) is
  software-pipelined: emitted after the NEXT pass's first big Exp, so
  ACT never stalls waiting for the DVE chain, and the tiny pos/epos/out
  DMAs ride the gpsimd queue so the in-order sync queue only carries
  the two 2 MiB streaming loads per pass.

Sharding: data-parallel over B. Each core gets 256 rows (2 partition
groups of 128), computes the partial sum of (lse - pos_logit) over its
2048 (row, positive) pairs, and writes one f32 scalar. The host sums
the 8 partials and divides by B*P.
"""

import sys

import numpy as np

sys.path.insert(0, "/opt/trn_rl_repo")

import jax

jax.config.update("jax_compilation_cache_dir", "/tmp/jax_bass_cache")
jax.config.update("jax_persistent_cache_min_compile_time_secs", 0.0)
jax.config.update("jax_persistent_cache_min_entry_size_bytes", 0)

import concourse.bacc as bacc
import concourse.bass as bass
import concourse.bass2jax as bass2jax
import concourse.mybir as mybir
from concourse import tile
from concourse.bass_utils import compile_bir_kernel as _orig_compile_bir_kernel
from concourse.bass_utils import run_bass_kernel_spmd

# NEFF compile memoization: walrus/neuronx-cc takes minutes per compile and
# this path has no cache of its own. Keyed on the BIR JSON content hash.
_NEFF_CACHE_DIR = "/tmp/neff_cache"


def _cached_compile_bir_kernel(bir_json, tmpdir, neff_name="file.neff"):
    import hashlib
    import os
    import shutil

    os.makedirs(_NEFF_CACHE_DIR, exist_ok=True)
    h = hashlib.sha256(bir_json).hexdigest()[:32]
    cpath = os.path.join(_NEFF_CACHE_DIR, h + ".neff")
    if os.path.exists(cpath):
        dst = os.path.join(tmpdir, neff_name)
        shutil.copy(cpath, dst)
        return dst
    p = _orig_compile_bir_kernel(bir_json, tmpdir, neff_name)
    shutil.copy(p, cpath + ".tmp")
    os.replace(cpath + ".tmp", cpath)
    return p


bass2jax.compile_bir_kernel = _cached_compile_bir_kernel

# Both Exp and Ln live in the natural_log_exp_and_others ACT table set, but
# the table-load placement pass greedily assigns Exp to exp_and_others and
# Ln to the ln set, thrashing two ~1.3us ACT table loads per pass. Restrict
# Exp/Ln membership to the combined set (names/indices preserved) so one
# load at kernel start covers everything.
from concourse.hw_specs import get_activation_tables as _orig_gat


def _gat_single_set(module_arch):
    AF = mybir.ActivationFunctionType
    out = {}
    for name, funcs in _orig_gat(module_arch).items():
        if name != "natural_log_exp_and_others":
            funcs = funcs - {AF.Exp, AF.Ln}
        out[name] = set(funcs)
    return out


bacc.get_activation_tables = _gat_single_set

B, C, P = 2048, 8192, 8
NCORES = 8
RB = B // NCORES          # 256 rows per core
G = RB // 128             # 2 partition groups of 128 rows
F32 = mybir.dt.float32
BF16 = mybir.dt.bfloat16

_NC = None


def _build_nc(repeat=1):
    nc = bacc.Bacc("TRN2", target_bir_lowering=False, debug=False, num_devices=NCORES)

    preds = nc.dram_tensor("preds", [RB, C], BF16, kind="ExternalInput")
    posd = nc.dram_tensor("pos", [128, G * P], F32, kind="ExternalInput")
    eposd = nc.dram_tensor("epos", [128, G * P], F32, kind="ExternalInput")
    out = nc.dram_tensor("partial", [1, 1], F32, kind="ExternalOutput")

    AF = mybir.ActivationFunctionType
    AX = mybir.AxisListType

    with tile.TileContext(nc) as tc:
        with (
            tc.tile_pool(name="io", bufs=4) as io,
            tc.tile_pool(name="small", bufs=3) as small,
            tc.tile_pool(name="const", bufs=1) as const,
            tc.tile_pool(name="ps", bufs=2, space="PSUM") as ps,
        ):
            ones = const.tile([128, 1], F32)
            nc.vector.memset(ones[:], 1.0)

            def make_tail(pos_sb, a):
                def tail():
                    lse = small.tile([128, G * P], F32, tag="lse")
                    nc.scalar.activation(out=lse[:], in_=a[:], func=AF.Ln)
                    d = small.tile([128, G * P], F32, tag="d")
                    nc.vector.tensor_sub(out=d[:], in0=lse[:], in1=pos_sb[:])
                    rtot = small.tile([128, 1], F32, tag="rtot")
                    nc.vector.reduce_sum(out=rtot[:], in_=d[:], axis=AX.X)
                    acc = ps.tile([1, 1], F32, tag="acc")
                    nc.tensor.matmul(
                        out=acc[:], lhsT=rtot[:], rhs=ones[:], start=True, stop=True
                    )
                    res = small.tile([1, 1], F32, tag="res")
                    nc.vector.tensor_copy(out=res[:], in_=acc[:])
                    nc.gpsimd.dma_start(out=out[:], in_=res[:])

                return tail

            pend = None
            for _rep in range(repeat):
                pos_sb = small.tile([128, G * P], F32, tag="pos")
                nc.gpsimd.dma_start(out=pos_sb[:], in_=posd[:])
                e = small.tile([128, G * P], F32, tag="e")
                nc.gpsimd.dma_start(out=e[:], in_=eposd[:])
                stats = small.tile([128, G], F32, tag="stats")
                a = small.tile([128, G * P], F32, tag="a")

                for g in range(G):
                    x = io.tile([128, C], BF16, tag="x")
                    nc.sync.dma_start(
                        out=x[:], in_=preds[g * 128 : (g + 1) * 128, :]
                    )
                    nc.scalar.activation(
                        out=x[:],
                        in_=x[:],
                        func=AF.Exp,
                        accum_out=stats[:, g : g + 1],
                    )
                    if g == 0 and pend is not None:
                        # previous pass's tail lands on ACT after this
                        # pass's first big Exp: no ACT stall on the DVE
                        # chain, and its out-DMA rides the gpsimd queue.
                        pend()
                    gp = slice(g * P, (g + 1) * P)
                    se = small.tile([128, 1], F32, tag="se")
                    nc.vector.reduce_sum(out=se[:], in_=e[:, gp], axis=AX.X)
                    en = small.tile([128, 1], F32, tag="en")
                    nc.vector.tensor_sub(out=en[:], in0=stats[:, g : g + 1], in1=se[:])
                    nc.vector.tensor_scalar_add(out=a[:, gp], in0=e[:, gp], scalar1=en[:])

                pend = make_tail(pos_sb, a)
            pend()

    nc.finalize()
    return nc


def _to_bf16(a):
    """f32 -> bf16 with round-to-nearest-even, vectorized."""
    u = a.view(np.uint32)
    rounded = u + 0x7FFF + ((u >> 16) & 1)
    return (rounded >> 16).astype(np.uint16)


def _make_in_maps(predictions, labels):
    import ml_dtypes

    preds_full = np.ascontiguousarray(np.asarray(predictions, dtype=np.float32))
    labels_full = np.asarray(labels).astype(np.int64)
    bf_full = _to_bf16(preds_full).view(ml_dtypes.bfloat16)
    # positive logits in f32, laid out pos[part, g*P+q]
    pos_full = np.take_along_axis(preds_full, labels_full, axis=1)  # [B, P] f32
    epos_full = np.exp(pos_full)
    in_maps = []
    for m in range(NCORES):
        sl = slice(m * RB, (m + 1) * RB)
        p = np.ascontiguousarray(bf_full[sl])

        def _fold(full):
            return np.ascontiguousarray(
                full[sl].reshape(G, 128, P).transpose(1, 0, 2).reshape(128, G * P)
            ).astype(np.float32)

        in_maps.append({"preds": p, "pos": _fold(pos_full), "epos": _fold(epos_full)})
    return in_maps


def kernel(predictions, labels):
    global _NC
    if _NC is None:
        _NC = _build_nc()
    in_maps = _make_in_maps(predictions, labels)
    res = run_bass_kernel_spmd(_NC, in_maps, list(range(NCORES))).results
    total = float(sum(float(r["partial"][0, 0]) for r in res))
    return np.asarray(total / (B * P), dtype=np.float32)


# revision 6
# speedup vs baseline: 1.0553x; 1.0553x over previous
"""Multi-label softmax cross-entropy loss on 8 Trainium2 NeuronCores.

Math (per row b with positives l_1..l_P, unique):
    T   = sum_c exp(pred[b,c])              (all classes)
    e_q = exp(pred[b,l_q])                  (each positive)
    En  = T - sum_q e_q                     (negatives only)
    lse_p = log(En + e_p)
    loss  = mean over (b,p) of (lse_p - pred[b,l_p])

No max-shift is needed: inputs are standard-normal so exp() stays well
inside f32 range (sum ~ 1.4e4).

Design (vs the f32 indirect-gather baseline at ~24.2 us/pass):
- predictions stream in as bf16 (host round-to-nearest-even convert).
  HBM traffic halves (8 MiB -> 4 MiB per core), moving the bottleneck
  from DMA (~23.4 us) to the ACT exp pass: 2 groups x 8192 elems/
  partition at 1 elem/cycle/lane @ 1.2 GHz ~= 13.7 us busy (measured
  ~14.5 with instruction overheads; ACT rate is dtype-independent, so
  fp8 would buy nothing and bf16 keeps accuracy at ~5e-7).
- positive logits AND their exps are computed on the host in f32
  (16 K elements vs 16.8 M on device) and passed as two tiny [128,16]
  inputs. This removes 16 serial indirect DMAs and one ACT op; using
  f32 instead of bf16-quantized positives shifts En by ~2e-6 rel.
- ACT work per pass is exactly: Exp(8192) x2 + Ln(16). Exp and Ln live
  in one table set (natural_log_exp_and_others): one load, no
  mid-kernel switches.
- The tail (Ln -> d -> row-sum -> 128-row matmul reduce -> out DMA) is
  software-pipelined: emitted after the NEXT pass's first big Exp, so
  ACT never stalls waiting for the DVE chain, and the tiny pos/epos/out
  DMAs ride the gpsimd queue so the in-order sync queue only carries
  the two 2 MiB streaming loads per pass.

Sharding: data-parallel over B. Each core gets 256 rows (2 partition
groups of 128), computes the partial sum of (lse - pos_logit) over its
2048 (row, positive) pairs, and writes one f32 scalar. The host sums
the 8 partials and divides by B*P.
"""

import sys

import numpy as np

sys.path.insert(0, "/opt/trn_rl_repo")

import jax

jax.config.update("jax_compilation_cache_dir", "/tmp/jax_bass_cache")
jax.config.update("jax_persistent_cache_min_compile_time_secs", 0.0)
jax.config.update("jax_persistent_cache_min_entry_size_bytes", 0)

import concourse.bacc as bacc
import concourse.bass2jax as bass2jax
import concourse.mybir as mybir
from concourse import tile
from concourse.bass_utils import compile_bir_kernel as _orig_compile_bir_kernel
from concourse.bass_utils import run_bass_kernel_spmd

# NEFF compile memoization: walrus/neuronx-cc takes minutes per compile and
# this path has no cache of its own. Keyed on the BIR JSON content hash.
_NEFF_CACHE_DIR = "/tmp/neff_cache"


def _cached_compile_bir_kernel(bir_json, tmpdir, neff_name="file.neff"):
    import hashlib
    import os
    import shutil

    os.makedirs(_NEFF_CACHE_DIR, exist_ok=True)
    h = hashlib.sha256(bir_json).hexdigest()[:32]
    cpath = os.path.join(_NEFF_CACHE_DIR, h + ".neff")
    if os.path.exists(cpath):
        dst = os.path.join(tmpdir, neff_name)
        shutil.copy(cpath, dst)
        return dst
    p = _orig_compile_bir_kernel(bir_json, tmpdir, neff_name)
    shutil.copy(p, cpath + ".tmp")
    os.replace(cpath + ".tmp", cpath)
    return p


bass2jax.compile_bir_kernel = _cached_compile_bir_kernel

# Both Exp and Ln live in the natural_log_exp_and_others ACT table set, but
# the table-load placement pass greedily assigns Exp to exp_and_others and
# Ln to the ln set, thrashing two ~1.3us ACT table loads per pass. Restrict
# Exp/Ln membership to the combined set (names/indices preserved) so one
# load at kernel start covers everything.
from concourse.hw_specs import get_activation_tables as _orig_gat


def _gat_single_set(module_arch):
    AF = mybir.ActivationFunctionType
    out = {}
    for name, funcs in _orig_gat(module_arch).items():
        if name != "natural_log_exp_and_others":
            funcs = funcs - {AF.Exp, AF.Ln}
        out[name] = set(funcs)
    return out


bacc.get_activation_tables = _gat_single_set

B, C, P = 2048, 8192, 8
NCORES = 8
RB = B // NCORES          # 256 rows per core
G = RB // 128             # 2 partition groups of 128 rows
F32 = mybir.dt.float32
BF16 = mybir.dt.bfloat16

_NC = None


def _build_nc(repeat=1):
    nc = bacc.Bacc("TRN2", target_bir_lowering=False, debug=False, num_devices=NCORES)

    preds = nc.dram_tensor("preds", [RB, C], BF16, kind="ExternalInput")
    posd = nc.dram_tensor("pos", [128, G * P], F32, kind="ExternalInput")
    eposd = nc.dram_tensor("epos", [128, G * P], F32, kind="ExternalInput")
    out = nc.dram_tensor("partial", [1, 1], F32, kind="ExternalOutput")

    AF = mybir.ActivationFunctionType
    AX = mybir.AxisListType

    with tile.TileContext(nc) as tc:
        with (
            tc.tile_pool(name="io", bufs=4) as io,
            tc.tile_pool(name="small", bufs=3) as small,
            tc.tile_pool(name="const", bufs=1) as const,
            tc.tile_pool(name="ps", bufs=2, space="PSUM") as ps,
        ):
            ones = const.tile([128, 1], F32)
            nc.vector.memset(ones[:], 1.0)

            def make_tail(pos_sb, a):
                def tail():
                    lse = small.tile([128, G * P], F32, tag="lse")
                    nc.scalar.activation(out=lse[:], in_=a[:], func=AF.Ln)
                    d = small.tile([128, G * P], F32, tag="d")
                    nc.vector.tensor_sub(out=d[:], in0=lse[:], in1=pos_sb[:])
                    rtot = small.tile([128, 1], F32, tag="rtot")
                    nc.vector.reduce_sum(out=rtot[:], in_=d[:], axis=AX.X)
                    acc = ps.tile([1, 1], F32, tag="acc")
                    nc.tensor.matmul(
                        out=acc[:], lhsT=rtot[:], rhs=ones[:], start=True, stop=True
                    )
                    res = small.tile([1, 1], F32, tag="res")
                    nc.vector.tensor_copy(out=res[:], in_=acc[:])
                    nc.gpsimd.dma_start(out=out[:], in_=res[:])

                return tail

            pend = None
            for _rep in range(repeat):
                pos_sb = small.tile([128, G * P], F32, tag="pos")
                nc.gpsimd.dma_start(out=pos_sb[:], in_=posd[:])
                e = small.tile([128, G * P], F32, tag="e")
                nc.gpsimd.dma_start(out=e[:], in_=eposd[:])
                stats = small.tile([128, G], F32, tag="stats")
                a = small.tile([128, G * P], F32, tag="a")

                for g in range(G):
                    x = io.tile([128, C], BF16, tag="x")
                    nc.sync.dma_start(
                        out=x[:], in_=preds[g * 128 : (g + 1) * 128, :]
                    )
                    nc.scalar.activation(
                        out=x[:],
                        in_=x[:],
                        func=AF.Exp,
                        accum_out=stats[:, g : g + 1],
                    )
                    if g == 0 and pend is not None:
                        # previous pass's tail lands on ACT after this
                        # pass's first big Exp: no ACT stall on the DVE
                        # chain, and its out-DMA rides the gpsimd queue.
                        pend()
                    gp = slice(g * P, (g + 1) * P)
                    se = small.tile([128, 1], F32, tag="se")
                    nc.vector.reduce_sum(out=se[:], in_=e[:, gp], axis=AX.X)
                    en = small.tile([128, 1], F32, tag="en")
                    nc.vector.tensor_sub(out=en[:], in0=stats[:, g : g + 1], in1=se[:])
                    nc.vector.tensor_scalar_add(out=a[:, gp], in0=e[:, gp], scalar1=en[:])

                pend = make_tail(pos_sb, a)
            pend()

    nc.finalize()
    return nc


def _to_bf16(a):
    """f32 -> bf16 with round-to-nearest-even, vectorized."""
    u = a.view(np.uint32)
    rounded = u + 0x7FFF + ((u >> 16) & 1)
    return (rounded >> 16).astype(np.uint16)


def _make_in_maps(predictions, labels):
    import ml_dtypes

    preds_full = np.ascontiguousarray(np.asarray(predictions, dtype=np.float32))
    labels_full = np.asarray(labels).astype(np.int64)
    bf_full = _to_bf16(preds_full).view(ml_dtypes.bfloat16)
    # positive logits in f32, laid out pos[part, g*P+q]
    pos_full = np.take_along_axis(preds_full, labels_full, axis=1)  # [B, P] f32
    epos_full = np.exp(pos_full)
    in_maps = []
    for m in range(NCORES):
        sl = slice(m * RB, (m + 1) * RB)
        p = np.ascontiguousarray(bf_full[sl])

        def _fold(full):
            return np.ascontiguousarray(
                full[sl].reshape(G, 128, P).transpose(1, 0, 2).reshape(128, G * P)
            ).astype(np.float32)

        in_maps.append({"preds": p, "pos": _fold(pos_full), "epos": _fold(epos_full)})
    return in_maps


def kernel(predictions, labels):
    global _NC
    if _NC is None:
        _NC = _build_nc()
    in_maps = _make_in_maps(predictions, labels)
    res = run_bass_kernel_spmd(_NC, in_maps, list(range(NCORES))).results
    total = float(sum(float(r["partial"][0, 0]) for r in res))
    return np.asarray(total / (B * P), dtype=np.float32)


# revision 7
# speedup vs baseline: 1.1204x; 1.0616x over previous
"""Multi-label softmax cross-entropy loss on 8 Trainium2 NeuronCores.

Math (per row b with positives l_1..l_P, unique):
    T   = sum_c exp(pred[b,c])              (all classes)
    e_q = exp(pred[b,l_q])                  (each positive)
    En  = T - sum_q e_q                     (negatives only)
    lse_p = log(En + e_p)
    loss  = mean over (b,p) of (lse_p - pred[b,l_p])

No max-shift is needed: inputs are standard-normal so exp() stays well
inside f32 range (sum ~ 1.4e4).

Design (vs the f32 indirect-gather baseline at ~24.2 us/pass):
- predictions stream in as bf16 (host round-to-nearest-even convert).
  HBM traffic halves (8 MiB -> 4 MiB per core), moving the bottleneck
  from DMA (~23.4 us) to the ACT exp pass: 2 groups x 8192 elems/
  partition at 1 elem/cycle/lane @ 1.2 GHz ~= 13.7 us busy (measured
  ~14.5 with instruction overheads; ACT rate is dtype-independent, so
  fp8 would buy nothing and bf16 keeps accuracy at ~5e-7).
- positive logits AND their exps are computed on the host in f32
  (16 K elements vs 16.8 M on device) and passed as two tiny [128,16]
  inputs. This removes 16 serial indirect DMAs and one ACT op; using
  f32 instead of bf16-quantized positives shifts En by ~2e-6 rel.
- ACT work per pass is exactly: Exp(8192) x2 + Ln(16). Exp and Ln live
  in one table set (natural_log_exp_and_others): one load, no
  mid-kernel switches.
- The tail (Ln -> d -> row-sum -> 128-row matmul reduce -> out DMA) is
  software-pipelined: emitted after the NEXT pass's first big Exp, so
  ACT never stalls waiting for the DVE chain, and the tiny pos/epos/out
  DMAs ride the gpsimd queue so the in-order sync queue only carries
  the two 2 MiB streaming loads per pass.

Sharding: data-parallel over B. Each core gets 256 rows (2 partition
groups of 128), computes the partial sum of (lse - pos_logit) over its
2048 (row, positive) pairs, and writes one f32 scalar. The host sums
the 8 partials and divides by B*P.
"""

import sys

import numpy as np

sys.path.insert(0, "/opt/trn_rl_repo")

import jax

jax.config.update("jax_compilation_cache_dir", "/tmp/jax_bass_cache")
jax.config.update("jax_persistent_cache_min_compile_time_secs", 0.0)
jax.config.update("jax_persistent_cache_min_entry_size_bytes", 0)

import concourse.bacc as bacc
import concourse.bass2jax as bass2jax
import concourse.mybir as mybir
from concourse import tile
from concourse.bass_utils import compile_bir_kernel as _orig_compile_bir_kernel
from concourse.bass_utils import run_bass_kernel_spmd

# NEFF compile memoization: walrus/neuronx-cc takes minutes per compile and
# this path has no cache of its own. Keyed on the BIR JSON content hash.
_NEFF_CACHE_DIR = "/tmp/neff_cache"


def _cached_compile_bir_kernel(bir_json, tmpdir, neff_name="file.neff"):
    import hashlib
    import os
    import shutil

    os.makedirs(_NEFF_CACHE_DIR, exist_ok=True)
    h = hashlib.sha256(bir_json).hexdigest()[:32]
    cpath = os.path.join(_NEFF_CACHE_DIR, h + ".neff")
    if os.path.exists(cpath):
        dst = os.path.join(tmpdir, neff_name)
        shutil.copy(cpath, dst)
        return dst
    p = _orig_compile_bir_kernel(bir_json, tmpdir, neff_name)
    shutil.copy(p, cpath + ".tmp")
    os.replace(cpath + ".tmp", cpath)
    return p


bass2jax.compile_bir_kernel = _cached_compile_bir_kernel

# Both Exp and Ln live in the natural_log_exp_and_others ACT table set, but
# the table-load placement pass greedily assigns Exp to exp_and_others and
# Ln to the ln set, thrashing two ~1.3us ACT table loads per pass. Restrict
# Exp/Ln membership to the combined set (names/indices preserved) so one
# load at kernel start covers everything.
from concourse.hw_specs import get_activation_tables as _orig_gat


def _gat_single_set(module_arch):
    AF = mybir.ActivationFunctionType
    out = {}
    for name, funcs in _orig_gat(module_arch).items():
        if name != "natural_log_exp_and_others":
            funcs = funcs - {AF.Exp, AF.Ln}
        out[name] = set(funcs)
    return out


bacc.get_activation_tables = _gat_single_set

B, C, P = 2048, 8192, 8
NCORES = 8
RB = B // NCORES          # 256 rows per core
G = RB // 128             # 2 partition groups of 128 rows
F32 = mybir.dt.float32
BF16 = mybir.dt.bfloat16
I32 = mybir.dt.int32
ALU = mybir.AluOpType

# DVE software-exp offload: the last WOFF columns of each group are
# exponentiated on the otherwise-idle vector engine (9 ALU passes:
# magic-number round, vertex-form quadratic for 2^f, exponent assembly
# in exact f32 arithmetic, final multiply with fused row-accumulate),
# shrinking the ACT activations that bound the kernel.
WOFF = 512
LOG2E = 1.4426950408889634
MAGIC = 12582912.0                      # 1.5 * 2^23
QA = 1.477315585
QC = 0.237349735
QD = 0.482345007


def _emit_exp_dve(nc, sm, x_ap, w, racc, tag=""):
    """DVE-only exp: racc[128,1] += sum over w of exp(x_ap). ~2.7e-3 rel."""
    y = sm.tile([128, w], F32, tag=f"dy{tag}")
    nc.vector.tensor_scalar(
        out=y[:], in0=x_ap, scalar1=LOG2E, scalar2=16.0, op0=ALU.mult, op1=ALU.add
    )
    t = sm.tile([128, w], F32, tag=f"dt{tag}")
    nc.vector.tensor_scalar_add(out=t[:], in0=y[:], scalar1=MAGIC)
    fn = sm.tile([128, w], F32, tag=f"dfn{tag}")
    nc.vector.tensor_scalar_sub(out=fn[:], in0=t[:], scalar1=MAGIC)
    u = sm.tile([128, w], F32, tag=f"du{tag}")
    nc.vector.scalar_tensor_tensor(
        out=u[:], in0=y[:], scalar=QA, in1=fn[:], op0=ALU.add, op1=ALU.subtract
    )
    v = sm.tile([128, w], F32, tag=f"dv{tag}")
    nc.vector.tensor_mul(out=v[:], in0=u[:], in1=u[:])
    wq = sm.tile([128, w], F32, tag=f"dw{tag}")
    nc.vector.tensor_scalar(
        out=wq[:], in0=v[:], scalar1=QC, scalar2=QD, op0=ALU.mult, op1=ALU.add
    )
    s = sm.tile([128, w], F32, tag=f"ds{tag}")
    nc.vector.tensor_scalar(
        out=s[:], in0=fn[:], scalar1=111.0, scalar2=8388608.0, op0=ALU.add, op1=ALU.mult
    )
    mi = sm.tile([128, w], I32, tag=f"dmi{tag}")
    nc.vector.tensor_copy(out=mi[:], in_=s[:])
    r = sm.tile([128, w], F32, tag=f"dr{tag}")
    nc.vector.scalar_tensor_tensor(
        out=r[:], in0=wq[:], scalar=1.0, in1=mi[:].bitcast(F32),
        op0=ALU.mult, op1=ALU.mult, accum_out=racc,
    )

_NC = None


def _build_nc(repeat=1):
    nc = bacc.Bacc("TRN2", target_bir_lowering=False, debug=False, num_devices=NCORES)

    preds = nc.dram_tensor("preds", [RB, C], BF16, kind="ExternalInput")
    posd = nc.dram_tensor("pos", [128, G * P], F32, kind="ExternalInput")
    eposd = nc.dram_tensor("epos", [128, G * P], F32, kind="ExternalInput")
    out = nc.dram_tensor("partial", [1, 1], F32, kind="ExternalOutput")

    AF = mybir.ActivationFunctionType
    AX = mybir.AxisListType

    with tile.TileContext(nc) as tc:
        with (
            tc.tile_pool(name="io", bufs=4) as io,
            tc.tile_pool(name="small", bufs=3) as small,
            tc.tile_pool(name="const", bufs=1) as const,
            tc.tile_pool(name="ps", bufs=2, space="PSUM") as ps,
        ):
            ones = const.tile([128, 1], F32)
            nc.vector.memset(ones[:], 1.0)

            def make_tail(pos_sb, a):
                def tail():
                    lse = small.tile([128, G * P], F32, tag="lse")
                    nc.scalar.activation(out=lse[:], in_=a[:], func=AF.Ln)
                    d = small.tile([128, G * P], F32, tag="d")
                    nc.vector.tensor_sub(out=d[:], in0=lse[:], in1=pos_sb[:])
                    rtot = small.tile([128, 1], F32, tag="rtot")
                    nc.vector.reduce_sum(out=rtot[:], in_=d[:], axis=AX.X)
                    acc = ps.tile([1, 1], F32, tag="acc")
                    nc.tensor.matmul(
                        out=acc[:], lhsT=rtot[:], rhs=ones[:], start=True, stop=True
                    )
                    res = small.tile([1, 1], F32, tag="res")
                    nc.vector.tensor_copy(out=res[:], in_=acc[:])
                    nc.gpsimd.dma_start(out=out[:], in_=res[:])

                return tail

            pend = None
            for _rep in range(repeat):
                pos_sb = small.tile([128, G * P], F32, tag="pos")
                nc.gpsimd.dma_start(out=pos_sb[:], in_=posd[:])
                e = small.tile([128, G * P], F32, tag="e")
                nc.gpsimd.dma_start(out=e[:], in_=eposd[:])
                stats = small.tile([128, G], F32, tag="stats")
                a = small.tile([128, G * P], F32, tag="a")

                for g in range(G):
                    x = io.tile([128, C], BF16, tag="x")
                    nc.sync.dma_start(
                        out=x[:], in_=preds[g * 128 : (g + 1) * 128, :]
                    )
                    nc.scalar.activation(
                        out=x[:, : C - WOFF],
                        in_=x[:, : C - WOFF],
                        func=AF.Exp,
                        accum_out=stats[:, g : g + 1],
                    )
                    if g == 0 and pend is not None:
                        # previous pass's tail lands on ACT after this
                        # pass's first big Exp: no ACT stall on the DVE
                        # chain, and its out-DMA rides the gpsimd queue.
                        pend()
                    dsum = small.tile([128, 1], F32, tag="dsum")
                    _emit_exp_dve(nc, small, x[:, C - WOFF :], WOFF, dsum[:])
                    gp = slice(g * P, (g + 1) * P)
                    se = small.tile([128, 1], F32, tag="se")
                    nc.vector.reduce_sum(out=se[:], in_=e[:, gp], axis=AX.X)
                    tg = small.tile([128, 1], F32, tag="tg")
                    nc.vector.tensor_add(out=tg[:], in0=stats[:, g : g + 1], in1=dsum[:])
                    en = small.tile([128, 1], F32, tag="en")
                    nc.vector.tensor_sub(out=en[:], in0=tg[:], in1=se[:])
                    nc.vector.tensor_scalar_add(out=a[:, gp], in0=e[:, gp], scalar1=en[:])

                pend = make_tail(pos_sb, a)
            pend()

    nc.finalize()
    return nc


def _to_bf16(a):
    """f32 -> bf16 with round-to-nearest-even, vectorized."""
    u = a.view(np.uint32)
    rounded = u + 0x7FFF + ((u >> 16) & 1)
    return (rounded >> 16).astype(np.uint16)


def _make_in_maps(predictions, labels):
    import ml_dtypes

    preds_full = np.ascontiguousarray(np.asarray(predictions, dtype=np.float32))
    labels_full = np.asarray(labels).astype(np.int64)
    bf_full = _to_bf16(preds_full).view(ml_dtypes.bfloat16)
    # positive logits in f32, laid out pos[part, g*P+q]
    pos_full = np.take_along_axis(preds_full, labels_full, axis=1)  # [B, P] f32
    epos_full = np.exp(pos_full)
    in_maps = []
    for m in range(NCORES):
        sl = slice(m * RB, (m + 1) * RB)
        p = np.ascontiguousarray(bf_full[sl])

        def _fold(full):
            return np.ascontiguousarray(
                full[sl].reshape(G, 128, P).transpose(1, 0, 2).reshape(128, G * P)
            ).astype(np.float32)

        in_maps.append({"preds": p, "pos": _fold(pos_full), "epos": _fold(epos_full)})
    return in_maps


def kernel(predictions, labels):
    global _NC
    if _NC is None:
        _NC = _build_nc()
    in_maps = _make_in_maps(predictions, labels)
    res = run_bass_kernel_spmd(_NC, in_maps, list(range(NCORES))).results
    total = float(sum(float(r["partial"][0, 0]) for r in res))
    return np.asarray(total / (B * P), dtype=np.float32)


# revision 8
# speedup vs baseline: 1.1851x; 1.0578x over previous
"""Multi-label softmax cross-entropy loss on 8 Trainium2 NeuronCores.

Math (per row b with positives l_1..l_P, unique):
    T   = sum_c exp(pred[b,c])              (all classes)
    e_q = exp(pred[b,l_q])                  (each positive)
    En  = T - sum_q e_q                     (negatives only)
    lse_p = log(En + e_p)
    loss  = mean over (b,p) of (lse_p - pred[b,l_p])

No max-shift is needed: inputs are standard-normal so exp() stays well
inside f32 range (sum ~ 1.4e4).

Design (vs the f32 indirect-gather baseline at ~24.2 us/pass):
- predictions stream in as bf16 (host round-to-nearest-even convert).
  HBM traffic halves (8 MiB -> 4 MiB per core), moving the bottleneck
  from DMA (~23.4 us) to the ACT exp pass: 2 groups x 8192 elems/
  partition at 1 elem/cycle/lane @ 1.2 GHz ~= 13.7 us busy (measured
  ~14.5 with instruction overheads; ACT rate is dtype-independent, so
  fp8 would buy nothing and bf16 keeps accuracy at ~5e-7).
- positive logits AND their exps are computed on the host in f32
  (16 K elements vs 16.8 M on device) and passed as two tiny [128,16]
  inputs. This removes 16 serial indirect DMAs and one ACT op; using
  f32 instead of bf16-quantized positives shifts En by ~2e-6 rel.
- ACT work per pass is exactly: Exp(8192) x2 + Ln(16). Exp and Ln live
  in one table set (natural_log_exp_and_others): one load, no
  mid-kernel switches.
- The tail (Ln -> d -> row-sum -> 128-row matmul reduce -> out DMA) is
  software-pipelined: emitted after the NEXT pass's first big Exp, so
  ACT never stalls waiting for the DVE chain, and the tiny pos/epos/out
  DMAs ride the gpsimd queue so the in-order sync queue only carries
  the two 2 MiB streaming loads per pass.

Sharding: data-parallel over B. Each core gets 256 rows (2 partition
groups of 128), computes the partial sum of (lse - pos_logit) over its
2048 (row, positive) pairs, and writes one f32 scalar. The host sums
the 8 partials and divides by B*P.
"""

import sys

import numpy as np

sys.path.insert(0, "/opt/trn_rl_repo")

import jax

jax.config.update("jax_compilation_cache_dir", "/tmp/jax_bass_cache")
jax.config.update("jax_persistent_cache_min_compile_time_secs", 0.0)
jax.config.update("jax_persistent_cache_min_entry_size_bytes", 0)

import concourse.bacc as bacc
import concourse.bass2jax as bass2jax
import concourse.mybir as mybir
from concourse import tile
from concourse.bass_utils import compile_bir_kernel as _orig_compile_bir_kernel
from concourse.bass_utils import run_bass_kernel_spmd

# NEFF compile memoization: walrus/neuronx-cc takes minutes per compile and
# this path has no cache of its own. Keyed on the BIR JSON content hash.
_NEFF_CACHE_DIR = "/tmp/neff_cache"


def _cached_compile_bir_kernel(bir_json, tmpdir, neff_name="file.neff"):
    import hashlib
    import os
    import shutil

    os.makedirs(_NEFF_CACHE_DIR, exist_ok=True)
    h = hashlib.sha256(bir_json).hexdigest()[:32]
    cpath = os.path.join(_NEFF_CACHE_DIR, h + ".neff")
    if os.path.exists(cpath):
        dst = os.path.join(tmpdir, neff_name)
        shutil.copy(cpath, dst)
        return dst
    p = _orig_compile_bir_kernel(bir_json, tmpdir, neff_name)
    shutil.copy(p, cpath + ".tmp")
    os.replace(cpath + ".tmp", cpath)
    return p


bass2jax.compile_bir_kernel = _cached_compile_bir_kernel

# Both Exp and Ln live in the natural_log_exp_and_others ACT table set, but
# the table-load placement pass greedily assigns Exp to exp_and_others and
# Ln to the ln set, thrashing two ~1.3us ACT table loads per pass. Restrict
# Exp/Ln membership to the combined set (names/indices preserved) so one
# load at kernel start covers everything.
from concourse.hw_specs import get_activation_tables as _orig_gat


def _gat_single_set(module_arch):
    AF = mybir.ActivationFunctionType
    out = {}
    for name, funcs in _orig_gat(module_arch).items():
        if name != "natural_log_exp_and_others":
            funcs = funcs - {AF.Exp, AF.Ln}
        out[name] = set(funcs)
    return out


bacc.get_activation_tables = _gat_single_set

B, C, P = 2048, 8192, 8
NCORES = 8
RB = B // NCORES          # 256 rows per core
G = RB // 128             # 2 partition groups of 128 rows
F32 = mybir.dt.float32
BF16 = mybir.dt.bfloat16
I32 = mybir.dt.int32
ALU = mybir.AluOpType

# DVE software-exp offload: the last WOFF columns of each group are
# exponentiated on the otherwise-idle vector engine (6 ALU passes:
# magic-number round, vertex-form quadratic for 2^f, exponent assembly
# in exact f32 arithmetic, final multiply with fused row-accumulate),
# shrinking the ACT activations that bound the kernel.
WOFF = 768
LOG2E = 1.4426950408889634
MAGIC = 12582912.0                      # 1.5 * 2^23
QA = 1.477315585
QC = 0.237349735
QD = 0.482345007


def _emit_exp_dve(nc, sm, x_ap, w, racc, tag=""):
    """DVE-only exp: racc[128,1] = sum over w of exp(x_ap). ~2.7e-3 rel.

    6 fused ALU passes; the two-op tensor_scalar rounds its intermediate
    to f32, which the magic-number rounding step relies on, and the
    exponent assembly converts on write to int32 (exact: (fn+111)*2^23
    has only 8 significant bits).
    """
    y = sm.tile([128, w], F32, tag=f"dy{tag}")
    nc.vector.tensor_scalar(
        out=y[:], in0=x_ap, scalar1=LOG2E, scalar2=16.0, op0=ALU.mult, op1=ALU.add
    )
    fn = sm.tile([128, w], F32, tag=f"dfn{tag}")
    nc.vector.tensor_scalar(
        out=fn[:], in0=y[:], scalar1=MAGIC, scalar2=MAGIC, op0=ALU.add, op1=ALU.subtract
    )
    u = sm.tile([128, w], F32, tag=f"du{tag}")
    nc.vector.scalar_tensor_tensor(
        out=u[:], in0=y[:], scalar=QA, in1=fn[:], op0=ALU.add, op1=ALU.subtract
    )
    w2 = sm.tile([128, w], F32, tag=f"dw2{tag}")
    nc.vector.scalar_tensor_tensor(
        out=w2[:], in0=u[:], scalar=QC, in1=u[:], op0=ALU.mult, op1=ALU.mult
    )
    mi = sm.tile([128, w], I32, tag=f"dmi{tag}")
    nc.vector.tensor_scalar(
        out=mi[:], in0=fn[:], scalar1=111.0, scalar2=8388608.0, op0=ALU.add, op1=ALU.mult
    )
    r = sm.tile([128, w], F32, tag=f"dr{tag}")
    nc.vector.scalar_tensor_tensor(
        out=r[:], in0=w2[:], scalar=QD, in1=mi[:].bitcast(F32),
        op0=ALU.add, op1=ALU.mult, accum_out=racc,
    )


_NC = None


def _build_nc(repeat=1):
    nc = bacc.Bacc("TRN2", target_bir_lowering=False, debug=False, num_devices=NCORES)

    preds = nc.dram_tensor("preds", [RB, C], BF16, kind="ExternalInput")
    posd = nc.dram_tensor("pos", [128, G * P], F32, kind="ExternalInput")
    eposd = nc.dram_tensor("epos", [128, G * P], F32, kind="ExternalInput")
    out = nc.dram_tensor("partial", [1, 1], F32, kind="ExternalOutput")

    AF = mybir.ActivationFunctionType
    AX = mybir.AxisListType

    with tile.TileContext(nc) as tc:
        with (
            tc.tile_pool(name="io", bufs=4) as io,
            tc.tile_pool(name="small", bufs=3) as small,
            tc.tile_pool(name="const", bufs=1) as const,
            tc.tile_pool(name="ps", bufs=2, space="PSUM") as ps,
        ):
            ones = const.tile([128, 1], F32)
            nc.vector.memset(ones[:], 1.0)

            def make_tail(pos_sb, a):
                def tail():
                    lse = small.tile([128, G * P], F32, tag="lse")
                    nc.scalar.activation(out=lse[:], in_=a[:], func=AF.Ln)
                    d = small.tile([128, G * P], F32, tag="d")
                    nc.vector.tensor_sub(out=d[:], in0=lse[:], in1=pos_sb[:])
                    rtot = small.tile([128, 1], F32, tag="rtot")
                    nc.vector.reduce_sum(out=rtot[:], in_=d[:], axis=AX.X)
                    acc = ps.tile([1, 1], F32, tag="acc")
                    nc.tensor.matmul(
                        out=acc[:], lhsT=rtot[:], rhs=ones[:], start=True, stop=True
                    )
                    res = small.tile([1, 1], F32, tag="res")
                    nc.vector.tensor_copy(out=res[:], in_=acc[:])
                    nc.gpsimd.dma_start(out=out[:], in_=res[:])

                return tail

            pend = None
            for _rep in range(repeat):
                pos_sb = small.tile([128, G * P], F32, tag="pos")
                nc.gpsimd.dma_start(out=pos_sb[:], in_=posd[:])
                e = small.tile([128, G * P], F32, tag="e")
                nc.gpsimd.dma_start(out=e[:], in_=eposd[:])
                stats = small.tile([128, G], F32, tag="stats")
                a = small.tile([128, G * P], F32, tag="a")

                for g in range(G):
                    x = io.tile([128, C], BF16, tag="x")
                    nc.sync.dma_start(
                        out=x[:], in_=preds[g * 128 : (g + 1) * 128, :]
                    )
                    nc.scalar.activation(
                        out=x[:, : C - WOFF],
                        in_=x[:, : C - WOFF],
                        func=AF.Exp,
                        accum_out=stats[:, g : g + 1],
                    )
                    if g == 0 and pend is not None:
                        # previous pass's tail lands on ACT after this
                        # pass's first big Exp: no ACT stall on the DVE
                        # chain, and its out-DMA rides the gpsimd queue.
                        pend()
                    dsum = small.tile([128, 1], F32, tag="dsum")
                    _emit_exp_dve(nc, small, x[:, C - WOFF :], WOFF, dsum[:])
                    gp = slice(g * P, (g + 1) * P)
                    se = small.tile([128, 1], F32, tag="se")
                    nc.vector.reduce_sum(out=se[:], in_=e[:, gp], axis=AX.X)
                    tg = small.tile([128, 1], F32, tag="tg")
                    nc.vector.tensor_add(out=tg[:], in0=stats[:, g : g + 1], in1=dsum[:])
                    en = small.tile([128, 1], F32, tag="en")
                    nc.vector.tensor_sub(out=en[:], in0=tg[:], in1=se[:])
                    nc.vector.tensor_scalar_add(out=a[:, gp], in0=e[:, gp], scalar1=en[:])

                pend = make_tail(pos_sb, a)
            pend()

    nc.finalize()
    return nc


def _to_bf16(a):
    """f32 -> bf16 with round-to-nearest-even, vectorized."""
    u = a.view(np.uint32)
    rounded = u + 0x7FFF + ((u >> 16) & 1)
    return (rounded >> 16).astype(np.uint16)


def _make_in_maps(predictions, labels):
    import ml_dtypes

    preds_full = np.ascontiguousarray(np.asarray(predictions, dtype=np.float32))
    labels_full = np.asarray(labels).astype(np.int64)
    bf_full = _to_bf16(preds_full).view(ml_dtypes.bfloat16)
    # positive logits in f32, laid out pos[part, g*P+q]
    pos_full = np.take_along_axis(preds_full, labels_full, axis=1)  # [B, P] f32
    epos_full = np.exp(pos_full)
    in_maps = []
    for m in range(NCORES):
        sl = slice(m * RB, (m + 1) * RB)
        p = np.ascontiguousarray(bf_full[sl])

        def _fold(full):
            return np.ascontiguousarray(
                full[sl].reshape(G, 128, P).transpose(1, 0, 2).reshape(128, G * P)
            ).astype(np.float32)

        in_maps.append({"preds": p, "pos": _fold(pos_full), "epos": _fold(epos_full)})
    return in_maps


def kernel(predictions, labels):
    global _NC
    if _NC is None:
        _NC = _build_nc()
    in_maps = _make_in_maps(predictions, labels)
    res = run_bass_kernel_spmd(_NC, in_maps, list(range(NCORES))).results
    total = float(sum(float(r["partial"][0, 0]) for r in res))
    return np.asarray(total / (B * P), dtype=np.float32)
